# revision 41
# baseline (speedup 1.0000x reference)
"""Trainium2 Bass kernel for nn_AudioNetwork_37512244363307.

Algorithm: the reference applies 4 sequential blocks of
  frame(hop 1024, win 2048) -> rfft -> per-(c,k) linear recurrence over
  frames -> irfft * hann -> overlap-add -> tanh(gain*x)
with identity channel mixing.  The per-channel transfer vectors are ~1%
sparse (<= 32 nonzero of 1025 coeffs), so each block reduces to:
  - forward: per hop-chunk j, a_j(k) = sum_n u_j[n] e^{-2pi i k n/2048}
    for the nonzero k only (matmul against a small DFT basis);
    S[i,k] = a_i(k) + (-1)^k a_{i+1}(k)
  - recurrence o[i] = (S[i] + o[i-1]) * t   (hardware tensor_tensor_scan)
  - synthesis: output chunk j = Ocat[j] @ G where Ocat stacks
    [Re o_j, Im o_j, Re o_{j-1}, Im o_{j-1}] and G folds the irfft basis,
    hann window and overlap-add of the two contributing frames.
Channels x batch are sharded over 8 NeuronCores (8 channels each); the
final sum over channels/blocks is accumulated on-core and reduced on host.
Matmuls run as float32r (full fp32 data, single-pass PE mode).
"""
import numpy as np

WS = 2048
STEP = 1024
NCOEF = WS // 2 + 1   # 1025
CPD = 64
NB = 4
B = 4
T = 131072
FRAMES = T // STEP    # 128
FR1 = FRAMES + 1      # 129: leading zero/reset column per batch
NK = 32               # padded nonzero-coeff slots per channel
NCORES = 8
CH_PER_CORE = CPD // NCORES  # 8
SUBS = STEP // 128    # 8


def _hann():
    return 0.5 * (1.0 - np.cos(2.0 * np.pi * np.arange(WS) / WS))


def _make_tables(transfers):
    """Host-precomputed DFT/synthesis bases, per (block, channel).

    Returns arrays shaped for direct DMA into SBUF tiles:
      fwdb (NB, CPD, 128, SUBS, 2*NK)  lhsT for forward DFT
      synb (NB, CPD, 128, SUBS, 128)   lhsT for synthesis
      tsg  (NB, CPD, 2*NK, 2)          col0 transfer value, col1 (-1)^k
    """
    H = _hann()
    n1 = np.arange(STEP)
    fwdb = np.zeros((NB, CPD, 128, SUBS, 2 * NK), np.float32)
    synb = np.zeros((NB, CPD, 128, SUBS, 128), np.float32)
    tsg = np.zeros((NB, CPD, 2 * NK, 2), np.float32)
    for i in range(NB):
        for c in range(CPD):
            t = transfers[i, c]
            ks = np.nonzero(t)[0]
            nk = len(ks)
            if nk > NK:
                raise ValueError("too many nonzero coeffs")
            kpad = np.zeros(NK, np.int64)
            kpad[:nk] = ks
            tpad = np.zeros(NK, np.float32)
            tpad[:nk] = t[ks]
            valid = np.zeros(NK, np.float32)
            valid[:nk] = 1.0
            th = 2.0 * np.pi * kpad[None, :] * n1[:, None] / WS  # (1024, NK)
            cos = np.cos(th) * valid
            sin = np.sin(th) * valid
            fwd = np.concatenate([cos, -sin], axis=1).astype(np.float32)
            fwdb[i, c] = fwd.reshape(SUBS, 128, 2 * NK).transpose(1, 0, 2)
            sign = np.where(kpad % 2 == 0, 1.0, -1.0).astype(np.float32)
            tsg[i, c, :NK, 0] = tpad
            tsg[i, c, NK:, 0] = tpad
            tsg[i, c, :NK, 1] = sign
            tsg[i, c, NK:, 1] = sign
            f = np.where(kpad == 0, 1.0 / WS, 2.0 / WS) * valid
            g1re = f[None, :] * H[:STEP, None] * np.cos(th)
            g1im = -f[None, :] * H[:STEP, None] * np.sin(th)
            g2re = f[None, :] * H[STEP:, None] * sign[None, :] * np.cos(th)
            g2im = -f[None, :] * H[STEP:, None] * sign[None, :] * np.sin(th)
            synth = np.concatenate(
                [g1re.T, g1im.T, g2re.T, g2im.T], axis=0).astype(np.float32)
            synb[i, c] = synth.reshape(128, SUBS, 128)
    return fwdb, synb, tsg


def _build_bass(gains, skew=True):
    import concourse.bass as bass
    import concourse.mybir as mybir
    from concourse import bacc, tile
    from collections import deque

    f32 = mybir.dt.float32
    f16 = mybir.dt.float16
    # fp16 basis blob per (block, channel): fwd lhsT [128,8,64] then synth
    # lhsT [128,8,128].  Loaded as one 1.5 MB DMA per (block, 4-channel
    # half), prefetched one half ahead so the PE never waits on it.
    BLOBW = 1536
    HCH = 4  # channels per blob DMA
    nc = bacc.Bacc()
    xin = nc.declare_dram_parameter(
        "xin", [128, CH_PER_CORE, SUBS, B, FRAMES], f16, isOutput=False)
    blob = nc.declare_dram_parameter(
        "blob", [NB, CH_PER_CORE // HCH, 128, HCH * BLOBW], f16,
        isOutput=False)
    ttsg = nc.declare_dram_parameter(
        "ttsg", [128, NB * (CH_PER_CORE // 2) * 2], f32, isOutput=False)
    outa = nc.declare_dram_parameter(
        "outa", [NB, 128, SUBS, B, FRAMES], f16, isOutput=True)
    outb = nc.declare_dram_parameter(
        "outb", [NB, 128, SUBS, B, FRAMES], f16, isOutput=True)
    outc = nc.declare_dram_parameter(
        "outc", [128, SUBS, B, FRAMES], f16, isOutput=True)
    wz = nc.declare_dram_parameter("wz", [128, 64], f16, isOutput=False)

    with tile.TileContext(nc) as tc:
        with (
            tc.tile_pool(name="res", bufs=CH_PER_CORE) as res_pool,
            tc.tile_pool(name="acc", bufs=2) as acc_pool,
            tc.tile_pool(name="blb", bufs=4) as blb_pool,
            tc.tile_pool(name="tg", bufs=4) as tg_pool,
            tc.tile_pool(name="work", bufs=6) as work_pool,
            tc.tile_pool(name="ones", bufs=1) as ones_pool,
            tc.tile_pool(name="fps", bufs=2, space=bass.MemorySpace.PSUM) as fps_pool,
            tc.tile_pool(name="sps", bufs=2, space=bass.MemorySpace.PSUM) as sps_pool,
        ):
            # sub-major layout: tanh writes and fwd matmul reads are
            # contiguous column ranges.
            res = [res_pool.tile([128, SUBS, B, FRAMES], f16, tag="res",
                                 name=f"res{c}")
                   for c in range(CH_PER_CORE)]
            # all-ones with the 4 per-batch reset columns zeroed; one
            # tensor_scalar against this materializes the scan's transfer
            # operand on-core.
            warm = work_pool.tile([128, 64], f16, tag="warm", bufs=1)
            warmps = fps_pool.tile([16, 64], f32, tag="fps")
            # zero via DMA: the gpsimd engine's init preamble (~10us) would
            # otherwise gate the PE warmup behind a memset
            nc.sync.dma_start(warm[:], wz[:])
            # pull the tanh spline table load (~2.7us) into the DMA-bound
            # startup window; tanh(0)=0 keeps `warm` zero.
            nc.scalar.activation(warm[:, 0:1], warm[:, 0:1],
                                 mybir.ActivationFunctionType.Tanh)
            for _ in range(60):
                nc.tensor.matmul(warmps[:], warm[:, 0:16], warm[:],
                                 start=True, stop=True)
            ones = ones_pool.tile([128, B, FR1], f32, tag="ones")
            nc.gpsimd.memset(ones[:], 1.0)
            nc.gpsimd.memset(ones[:, :, 0:1], 0.0)
            # all transfer/sign scalars in one tiny DMA, fetched up front
            tga = ones_pool.tile([128, NB * (CH_PER_CORE // 2) * 2], f32,
                                 tag="tga")
            nc.sync.dma_start(tga[:], ttsg[:])

            # prefetched blob tiles, one per (block, half): issued HCH
            # channels ahead of use.
            blob_tiles = {}

            # blob half layout: [fwd: HCH*512 | synth: HCH*1024].  Two SWDGE
            # DMAs per half (fwd lands first); bulk loads ride the gpsimd
            # ring so they never head-of-line-block the sync ring's
            # latency-critical ocat copies.
            FWDW = HCH * 512
            def fetch_blob(i, h, defer_syn=False):
                bt = blb_pool.tile([128, HCH * BLOBW], f16, tag="blb")
                nc.sync.dma_start(bt[:, 0:FWDW], blob[i, h, :, 0:FWDW])
                if not defer_syn:
                    nc.sync.dma_start(bt[:, FWDW:], blob[i, h, :, FWDW:])
                blob_tiles[(i, h)] = bt

            def fetch_blob_syn(i, h):
                bt = blob_tiles[(i, h)]
                nc.sync.dma_start(bt[:, FWDW:], blob[i, h, :, FWDW:])

            def front_half(i, p):
                """Paired front: channels (2p, 2p+1) share the fwd PSUM bank
                via column-group tiling, so S-build + scan run once per pair
                on all 128 partitions."""
                c0, c1 = 2 * p, 2 * p + 1
                if i == 0:
                    nc.sync.dma_start(res[c0][:], xin[:, c0])
                    nc.sync.dma_start(res[c1][:], xin[:, c1])
                    bt00 = blob_tiles[(0, 0)]
                    if p == 0:
                        # stage block-0 bases pairwise behind the inputs
                        # they serve, keeping the first tanh chain minimal
                        nc.sync.dma_start(
                            bt00[:, FWDW:FWDW + 2048],
                            blob[0, 0, :, FWDW:FWDW + 2048])
                        nc.sync.dma_start(
                            bt00[:, 1024:2048], blob[0, 0, :, 1024:2048])
                    elif p == 1:
                        nc.sync.dma_start(
                            bt00[:, FWDW + 2048:],
                            blob[0, 0, :, FWDW + 2048:])
                # prefetch: second half early in the block; next
                # block's halves during this block's back-drain window so
                # their transfers never delay latency-critical traffic.
                if p == 1 and (i, 1) not in blob_tiles:
                    fetch_blob(i, 1)
                elif p == 3 and i + 1 < NB:
                    fetch_blob(i + 1, 0)
                    fetch_blob(i + 1, 1)
                bt = blob_tiles[(i, p // 2)]
                ci = (p % 2) * 2
                bf0 = bt[:, ci * 512:(ci + 1) * 512]
                bf1 = bt[:, (ci + 1) * 512:(ci + 2) * 512]
                bs0 = bt[:, FWDW + ci * 1024:FWDW + (ci + 1) * 1024]
                bs1 = bt[:, FWDW + (ci + 1) * 1024:FWDW + (ci + 2) * 1024]
                tg = tga[:, (i * (CH_PER_CORE // 2) + p) * 2:
                         (i * (CH_PER_CORE // 2) + p) * 2 + 2]
                fb0 = bf0.rearrange('p (s m) -> p s m', s=SUBS)
                fb1 = bf1.rearrange('p (s m) -> p s m', s=SUBS)
                sb0 = bs0.rearrange('p (s m) -> p s m', s=SUBS)
                sb1 = bs1.rearrange('p (s m) -> p s m', s=SUBS)
                sg = tg[:, 1:2]

                fwdps = fps_pool.tile([128, B, FRAMES], f32, tag="fps")
                # (scheduler-priority boost for the last pair's DFT was
                # tried here and measured neutral; kept as a no-op)
                import contextlib
                prio = contextlib.nullcontext()
                with prio:
                    if i == 0:
                        # startup: group per channel so c0's DFT starts the
                        # moment its input lands, before c1's arrives.
                        for cg, fb, cc in ((0, fb0, c0), (64, fb1, c1)):
                            for s in range(SUBS):
                                nc.tensor.matmul(
                                    fwdps[cg:cg + 64], fb[:, s, :],
                                    res[cc][:, s, :, :],
                                    start=(s == 0), stop=(s == SUBS - 1),
                                    tile_position=(0, cg),
                                    skip_group_check=True)
                    else:
                        for s in range(SUBS):
                            nc.tensor.matmul(
                                fwdps[0:64], fb0[:, s, :],
                                res[c0][:, s, :, :],
                                start=(s == 0), stop=(s == SUBS - 1),
                                tile_position=(0, 0), skip_group_check=True)
                            nc.tensor.matmul(
                                fwdps[64:128], fb1[:, s, :],
                                res[c1][:, s, :, :],
                                start=(s == 0), stop=(s == SUBS - 1),
                                tile_position=(0, 64),
                                skip_group_check=True)
                # S-build + scan + o-concat: the latency-critical DVE
                # chain.  high_priority keeps the Tile ready-heap from ever
                # scheduling a (tanh-gated) channel-sum add ahead of it.
                with tc.high_priority(offset=1 << 20):
                    tt = work_pool.tile([128, B, FR1], f32, tag="tt")
                    nc.vector.tensor_scalar_mul(tt[:], ones[:], tg[:, 0:1])
                    stile = work_pool.tile([128, B, FR1], f32, tag="stile")
                    nc.gpsimd.memset(stile[:, :, 0:1], 0.0)
                    nc.vector.tensor_copy(
                        stile[:, :, 1:FR1], fwdps[:, :, 0:FRAMES])
                    nc.vector.scalar_tensor_tensor(
                        stile[:, :, 1:FRAMES], fwdps[:, :, 1:FRAMES], sg,
                        stile[:, :, 1:FRAMES],
                        mybir.AluOpType.mult, mybir.AluOpType.add)
                    # one batched scan: both channels (partition halves),
                    # all b; col (b,0) has t=0 so state resets at batch
                    # boundaries.
                    opair = work_pool.tile([128, B, FR1], f16, tag="opair")
                    nc.vector.tensor_tensor_scan(
                        opair[:].rearrange('p b j -> p (b j)'),
                        stile[:].rearrange('p b j -> p (b j)'),
                        tt[:].rearrange('p b j -> p (b j)'),
                        0.0, mybir.AluOpType.add, mybir.AluOpType.mult)
                    ocat0 = work_pool.tile([128, B, FR1], f16, tag="ocat0")
                    ocat1 = work_pool.tile([128, B, FR1], f16, tag="ocat1")
                    nc.vector.tensor_copy(
                        ocat0[0:64, :, 1:FR1], opair[0:64, :, 1:FR1])
                    nc.vector.tensor_copy(
                        ocat0[64:128, :, 1:FR1], opair[0:64, :, 0:FRAMES])
                    nc.vector.tensor_copy(
                        ocat1[0:64, :, 1:FR1], opair[64:128, :, 1:FR1])
                    nc.vector.tensor_copy(
                        ocat1[64:128, :, 1:FR1], opair[64:128, :, 0:FRAMES])
                return (sb0, ocat0), (sb1, ocat1)

            # channel-sum adds are deferred past the NEXT front's DVE ops:
            # an add waits on its tanh, and emitting it before the next
            # S-build would head-of-line-block the scan chain on the DVE.
            add_q = deque()

            def flush_adds():
                while add_q:
                    i, c, acca, accb = add_q.popleft()
                    acc = acca if c < 4 else accb
                    if c % 4 == 1:
                        nc.vector.tensor_add(acc[:], res[c - 1][:],
                                             res[c][:])
                    elif c % 4 != 0:
                        nc.vector.tensor_add(acc[:], acc[:], res[c][:])
                    last_b = CH_PER_CORE - 2 if i == NB - 1 \
                        else CH_PER_CORE - 1
                    if c == 3:
                        nc.sync.dma_start(outa[i], acca[:])
                    elif c == last_b:
                        nc.sync.dma_start(outb[i], accb[:])

            def back_half(i, c, sb, ocat, acca, accb):
                synrhs = ocat[:, :, 1:FR1]
                # tanh in 3-sub groups (PSUM free dim 1536/1536/1024):
                # fewer ACTIVATE calls amortize the per-call overhead.
                for s0, ns in ((0, 3), (3, 3), (6, 2)):
                    synps = sps_pool.tile([128, 3, B, FRAMES], f32,
                                          tag="sps")
                    for h in range(ns):
                        nc.tensor.matmul(
                            synps[:, h], sb[:, s0 + h, :], synrhs,
                            start=True, stop=True)
                    nc.scalar.activation(
                        res[c][:, s0:s0 + ns, :, :], synps[:, 0:ns],
                        mybir.ActivationFunctionType.Tanh,
                        scale=float(gains[i]))
                # two DVE-accumulated halves (shorter chains, host sums)
                if i == NB - 1 and c == CH_PER_CORE - 1:
                    # tail: ship the last channel raw; host folds it in
                    nc.sync.dma_start(outc[:], res[c][:])
                    return
                add_q.append((i, c, acca, accb))

            bt00 = blb_pool.tile([128, HCH * BLOBW], f16, tag="blb")
            nc.sync.dma_start(bt00[:, 0:1024], blob[0, 0, :, 0:1024])
            blob_tiles[(0, 0)] = bt00
            pend_q = deque()
            for i in range(NB):
                acca = acc_pool.tile([128, SUBS, B, FRAMES], f16, tag="acca")
                accb = acc_pool.tile([128, SUBS, B, FRAMES], f16, tag="accb")
                for p in range(CH_PER_CORE // 2):
                    st0, st1 = front_half(i, p)
                    flush_adds()
                    pend_q.append((i, 2 * p, st0[0], st0[1], acca, accb))
                    pend_q.append((i, 2 * p + 1, st1[0], st1[1], acca, accb))
                    while len(pend_q) > (4 if skew else 0):
                        pi, pc, psb, pocat, pacca, paccb = pend_q.popleft()
                        back_half(pi, pc, psb, pocat, pacca, paccb)
            while pend_q:
                pi, pc, psb, pocat, pacca, paccb = pend_q.popleft()
                back_half(pi, pc, psb, pocat, pacca, paccb)
            flush_adds()
    nc.compile()
    return nc


def _prep_inputs(x, transfers):
    fwdb, synb, tsg = _make_tables(transfers)
    fwdq = fwdb.reshape(NB, CPD, 128, 512).astype(np.float16)
    synq = synb.reshape(NB, CPD, 128, 1024).astype(np.float16)
    # (NB, CPD, 2NK, 2) -> pairwise stacked rows (NB, CPD//2, 128, 2)
    ttsg = tsg.reshape(NB, CPD // 2, 128, 2).astype(np.float32)
    # x (B, CPD, T) -> [n', c, b, s, j] with t = j*1024 + s*128 + n'
    x5 = x.reshape(B, CPD, FRAMES, SUBS, 128)
    xt = np.ascontiguousarray(
        np.transpose(x5, (4, 1, 3, 0, 2)).astype(np.float16))
    # half layout: [fwd: HCH*512 | syn pair0: 2048 | syn pair1: 2048]
    HCH = 4
    fwdpart = np.transpose(
        fwdq.reshape(NB, CPD // HCH, HCH, 128, 512), (0, 1, 3, 2, 4))
    synpart = np.transpose(
        synq.reshape(NB, CPD // HCH, HCH, 128, 1024), (0, 1, 3, 2, 4))
    blobr = np.concatenate(
        [fwdpart.reshape(NB, CPD // HCH, 128, HCH * 512),
         synpart.reshape(NB, CPD // HCH, 128, HCH * 1024)], axis=3)
    blobr = np.ascontiguousarray(blobr)
    in_maps = []
    for core in range(NCORES):
        cl = core * CH_PER_CORE
        ch = cl + CH_PER_CORE
        tcore = np.transpose(ttsg[:, cl // 2:ch // 2], (2, 0, 1, 3))
        in_maps.append({
            "xin": np.ascontiguousarray(xt[:, cl:ch]),
            "blob": np.ascontiguousarray(
                blobr[:, cl // HCH:ch // HCH]),
            "ttsg": np.ascontiguousarray(
                tcore.reshape(128, NB * (CPD // NCORES // 2) * 2)),
            "wz": np.zeros((128, 64), np.float16),
        })
    return in_maps


def _combine(x, outs, outcs, mixer):
    # outs: per-core list of (NB, 128, B, SUBS, FRAMES) block partials;
    # outcs: per-core raw last channel of the final block (tail shortcut)
    mv = np.exp(mixer - np.max(mixer))
    mv = (mv / mv.sum()).astype(np.float32)
    total = np.zeros((NB, 128, SUBS, B, FRAMES), np.float32)
    for o in outs:
        total += np.asarray(o, np.float32)
    for o in outcs:
        total[NB - 1] += np.asarray(o, np.float32)
    mixed = np.einsum('l...,l->...', total, mv[1:])  # (128, SUBS, B, FRAMES)
    y = np.transpose(mixed, (2, 3, 1, 0)).reshape(B, T)  # b, j, s, n'
    y = y + mv[0] * x.sum(axis=1)
    return np.ascontiguousarray(y[:, None, :]).astype(np.float32)


def _kernel_np_fallback(x, transfers, mixer_matrices, gains, mixer):
    H = _hann()
    frames = x.shape[-1] // STEP
    mv = np.exp(mixer - np.max(mixer))
    mv = mv / mv.sum()
    outputs = [x.astype(np.float32)]
    inp = x.astype(np.float32)
    idx = np.arange(frames)[:, None] * STEP + np.arange(WS)[None, :]
    for i in range(NB):
        xm = np.einsum('bct,cd->bdt', inp, mixer_matrices[i])
        xp = np.pad(xm, ((0, 0), (0, 0), (0, WS - STEP)))
        windowed = xp[..., idx]
        spec = np.fft.rfft(windowed, axis=-1)
        Tc = transfers[i].astype(spec.dtype)
        o = np.zeros(spec.shape[:2] + (spec.shape[3],), spec.dtype)
        outspec = np.empty_like(spec)
        for fidx in range(frames):
            o = (spec[:, :, fidx] + o) * Tc[None]
            outspec[:, :, fidx] = o
        wins = np.fft.irfft(outspec, n=WS, axis=-1) * H
        L = (frames - 1) * STEP + WS
        samples = np.zeros(xm.shape[:2] + (L,), np.float32)
        for fidx in range(frames):
            samples[..., fidx * STEP:fidx * STEP + WS] += \
                wins[:, :, fidx].astype(np.float32)
        inp = np.tanh(samples[..., :x.shape[-1]] * gains[i]).astype(np.float32)
        outputs.append(inp)
    result = np.stack(outputs, axis=-1)
    mixed = (result * mv[None, None, None, :]).sum(-1)
    return mixed.sum(axis=1, keepdims=True).astype(np.float32)


def _conforms(x, transfers, mixer_matrices, gains, mixer):
    try:
        if x.shape != (B, CPD, T) or transfers.shape != (NB, CPD, NCOEF):
            return False
        if mixer_matrices.shape != (NB, CPD, CPD) or gains.shape != (NB,):
            return False
        eye = np.eye(CPD, dtype=np.float32)
        if not all(np.array_equal(mixer_matrices[i], eye) for i in range(NB)):
            return False
        if (transfers != 0).sum(axis=-1).max() > NK:
            return False
        # k = WS/2 (Nyquist) term would need a different irfft scale
        if np.any(transfers[:, :, NCOEF - 1] != 0):
            return False
        return True
    except Exception:
        return False


_CACHE = {}


def kernel(**inputs):
    x = np.asarray(inputs["x"], np.float32)
    transfers = np.asarray(inputs["transfers"], np.float32)
    mixer_matrices = np.asarray(inputs["mixer_matrices"], np.float32)
    gains = np.asarray(inputs["gains"], np.float32)
    mixer = np.asarray(inputs["mixer"], np.float32)
    if not _conforms(x, transfers, mixer_matrices, gains, mixer):
        return _kernel_np_fallback(x, transfers, mixer_matrices, gains, mixer)

    from concourse.bass_utils import run_bass_kernel_spmd
    in_maps = _prep_inputs(x, transfers)
    key = gains.tobytes()
    if key not in _CACHE:
        _CACHE[key] = _build_bass(gains)
    nc = _CACHE[key]
    res = run_bass_kernel_spmd(nc, in_maps, list(range(NCORES)))
    outs = [res.results[i]["outa"] for i in range(NCORES)]
    outs += [res.results[i]["outb"] for i in range(NCORES)]
    outcs = [res.results[i]["outc"] for i in range(NCORES)]
    return _combine(x, outs, outcs, mixer)
